# revision 20
# baseline (speedup 1.0000x reference)
"""Trainium2 Bass kernel for nn_Criterion_32830730011569.

8 cores = (image b in 0..3) x (H-half h in 0..1). Each core streams its
[96,192] pixel slice once as a packed [NCHUNK,P,JC,256] (por||true) tensor
(fully linear 2.36MB chunk DMAs, prefetch depth 3).
  - dice: exp on ACT; per-row fused tensor_tensor_reduce does mask-mult +
    softmax-denominator in one DVE pass; true is cast to bf16 on ACT; bf16
    matmuls accumulate C = (true/Z)^T (exp*mask) into one [96,160] PSUM tile.
    num = 2*sum Hs*C; den = rowsums of C (masked-exp rows sum to Z).
  - 7x7-window BCE: true/bin shipped channel-major so each window row is a
    contiguous 7-float run gathered by indirect DMA (no extraction pass).
  - occupancy CE / class / NLL: slotted into DVE slack between dice chunks.
All small setup inputs ride in one packed [128,SC] tensor (single DMA).
Each core returns 7 partial sums; the host combines them into the loss.
"""
import sys

sys.path.insert(0, "/opt/trn_rl_repo")
import numpy as np

B, H, W, Q, E, M, K, WIN = 4, 192, 192, 160, 96, 96, 4, 7
NO_E = 0.1
HALF = H // 2          # rows per core slice
NPIX = HALF * W        # 18432 pixels per slice
P = 128                # partitions
J = NPIX // P          # 144 pixels per partition
NCHUNK = 8
JC = J // NCHUNK       # 18
CH = Q + E             # 256 packed channels (por | true)
MAGIC = 8388608.0      # 2^23
MAGIC_I = 0x4B000000
RUNW = 6 * W + 7       # one contiguous span covering a whole 7x7 window
PADF = RUNW            # front pad so straddle-up windows stay row-aligned
PADB = 2400

# smalls column map
C_ME, C_MQ, C_RB, C_INC, C_DROF, C_IOP96, C_IOPA, C_IOPB = 0, 1, 2, 3, 5, 12, 13, 14
C_POSA, C_POSB, C_CHOLA, C_CHOLB = 15, 17, 19, 23
C_MERR, C_MQRA, C_MQRB = 27, 123, 219
C_IOQ, C_IOE, C_IEL = 315, 475, 571
C_MENP, C_MQNP = 731, 732
SC = 733

_CACHE = {}


def _build_nc():
    import os
    import concourse.bass as bass
    import concourse.bacc as bacc
    import concourse.tile as tile
    from concourse import mybir

    DIS = set(os.environ.get("KDIS", "").split(","))

    f32 = mybir.dt.float32
    i32 = mybir.dt.int32
    bf16 = mybir.dt.bfloat16
    AF = mybir.ActivationFunctionType
    OP = mybir.AluOpType
    AX = mybir.AxisListType

    nc = bacc.Bacc("TRN2", target_bir_lowering=False, debug=False, num_devices=8)

    # ---- external I/O ----
    stream = nc.dram_tensor("stream", [NCHUNK * P, JC, CH], f32, kind="ExternalInput")
    occ_pack = nc.dram_tensor("occ_pack", [P, J, K + 1], f32, kind="ExternalInput")
    true_cm = nc.dram_tensor("true_cm", [1, PADF + E * NPIX + PADB], f32, kind="ExternalInput")
    bin_cm = nc.dram_tensor("bin_cm", [1, PADF + Q * NPIX + PADB], f32, kind="ExternalInput")
    smalls = nc.dram_tensor("smalls", [P, SC], f32, kind="ExternalInput")
    partials = nc.dram_tensor("partials", [1, 8], f32, kind="ExternalOutput")

    def bc(ap, pos, count):
        """Insert a stride-0 broadcast dim into an AP at free-dim position pos."""
        new = list(ap.ap)
        new.insert(pos, [0, count])
        return bass.AP(tensor=ap.tensor, offset=ap.offset, ap=new)

    from contextlib import ExitStack

    with tile.TileContext(nc) as tc, ExitStack() as ctx:
        sing = ctx.enter_context(tc.tile_pool(name="sing", bufs=1))
        pkp = ctx.enter_context(tc.tile_pool(name="pkp", bufs=3))
        big = ctx.enter_context(tc.tile_pool(name="big", bufs=2))
        ps = ctx.enter_context(tc.tile_pool(name="ps", bufs=1, space="PSUM"))

        # ---------- chunk prefetch (first instructions issued) ----------
        def issue_chunk(c, dep=None):
            from concourse.tile import add_dep_helper
            t = pkp.tile([P, JC, CH], f32, tag="pk")
            d = nc.sync.dma_start(out=t[:], in_=stream.ap()[c * P:(c + 1) * P, :, :])
            if dep is not None:
                # stagger: don't put this transfer on the wire until the prior
                # chunk's exp has started consuming, so in-flight chunks don't
                # interleave on the queue and delay each other
                add_dep_helper(d.ins, dep.ins, reason="stagger prefetch")
            return t

        # smalls/occ ride the Scalar engine's separate HWDGE queue so they are
        # not serialized behind the multi-MB chunk stream on the Sync queue
        sm = sing.tile([P, SC], f32)
        nc.scalar.dma_start(out=sm[:], in_=smalls.ap())
        occ_t = sing.tile([P, J, K + 1], f32)
        nc.scalar.dma_start(out=occ_t[:], in_=occ_pack.ap())
        pk_fifo = [issue_chunk(0), issue_chunk(1)]

        def S(p0, p1, c0, c1):
            return sm[p0:p1, c0:c1]

        def emit_ln(pref, out, x, pp, ff):
            """out = ln(x) for positive normal floats.

            Bit-extract exponent/mantissa, 3-term series on the reduced
            mantissa, then 2 Newton steps y += x*e^-y - 1 via the ACT Exp.
            """
            LN2 = 0.6931471805599453
            SQRT2 = 1.4142135623730951

            def T(nm, dt=f32):
                return sing.tile([pp, ff], dt, name=f"{pref}_{nm}", tag=f"{pref}_{nm}")

            xb = x.bitcast(i32)
            ei = T("ei", i32)
            nc.vector.tensor_scalar(out=ei[:], in0=xb, scalar1=23, scalar2=MAGIC_I,
                                    op0=OP.arith_shift_right, op1=OP.bitwise_or)
            ef = T("ef")
            nc.vector.tensor_scalar(out=ef[:], in0=ei[:].bitcast(f32),
                                    scalar1=-(MAGIC + 127.0), scalar2=None, op0=OP.add)
            mi = T("mi", i32)
            nc.vector.tensor_scalar(out=mi[:], in0=xb, scalar1=0x007FFFFF,
                                    scalar2=0x3F800000, op0=OP.bitwise_and, op1=OP.bitwise_or)
            mf = mi[:].bitcast(f32)
            cf = T("cf")
            nc.vector.tensor_scalar(out=cf[:], in0=mf, scalar1=SQRT2, scalar2=None, op0=OP.is_ge)
            hf = T("hf")
            nc.vector.tensor_scalar(out=hf[:], in0=cf[:], scalar1=-0.5, scalar2=1.0,
                                    op0=OP.mult, op1=OP.add)
            u = T("u")
            nc.vector.tensor_tensor(out=u[:], in0=mf, in1=hf[:], op=OP.mult)
            nc.vector.tensor_tensor(out=ef[:], in0=ef[:], in1=cf[:], op=OP.add)
            nc.vector.tensor_scalar(out=u[:], in0=u[:], scalar1=-1.0, scalar2=None, op0=OP.add)
            v = T("v")
            nc.vector.tensor_scalar(out=v[:], in0=u[:], scalar1=-0.25, scalar2=1.0 / 3.0,
                                    op0=OP.mult, op1=OP.add)
            nc.vector.tensor_tensor(out=v[:], in0=v[:], in1=u[:], op=OP.mult)
            nc.vector.tensor_scalar(out=v[:], in0=v[:], scalar1=-0.5, scalar2=None, op0=OP.add)
            nc.vector.tensor_tensor(out=v[:], in0=v[:], in1=u[:], op=OP.mult)
            nc.vector.tensor_scalar(out=v[:], in0=v[:], scalar1=1.0, scalar2=None, op0=OP.add)
            nc.vector.tensor_tensor(out=v[:], in0=v[:], in1=u[:], op=OP.mult)
            y = out
            nc.vector.tensor_scalar(out=y, in0=ef[:], scalar1=LN2, scalar2=None, op0=OP.mult)
            nc.vector.tensor_tensor(out=y, in0=y, in1=v[:], op=OP.add)
            ey = T("ey")
            w = T("w")
            for _ in range(2):
                nc.scalar.activation(out=ey[:], in_=y, func=AF.Exp, scale=-1.0)
                nc.vector.tensor_tensor(out=w[:], in0=ey[:], in1=x, op=OP.mult)
                nc.vector.tensor_scalar(out=w[:], in0=w[:], scalar1=-1.0, scalar2=None, op0=OP.add)
                nc.vector.tensor_tensor(out=y, in0=y, in1=w[:], op=OP.add)

        def emit_softplus(pref, out, x, pp, ff):
            """out = ln(1 + exp(x)) (inputs are O(1) logits, no overflow)."""
            opx = sing.tile([pp, ff], f32, name=pref + "_opx", tag=pref + "_opx")
            nc.scalar.activation(out=opx[:], in_=x, func=AF.Exp)
            nc.vector.tensor_scalar(out=opx[:], in0=opx[:], scalar1=1.0, scalar2=None, op0=OP.add)
            emit_ln(pref, out, opx[:], pp, ff)

        ones = sing.tile([P, 1], f32)
        nc.vector.memset(ones[:], 1.0)
        onesw = sing.tile([E, P], f32)
        nc.vector.memset(onesw[:], 1.0)
        stats = sing.tile([P, 6], f32)
        nc.vector.memset(stats[:], 0.0)
        res = sing.tile([1, 8], f32)
        nc.vector.memset(res[:], 0.0)

        # ---------- one-hot selectors (from smalls) ----------
        Mq = sing.tile([M, Q], f32)
        nc.vector.tensor_scalar(out=Mq[:], in0=S(0, M, C_IOQ, C_IOQ + Q),
                                scalar1=S(0, M, C_MQ, C_MQ + 1), scalar2=None, op0=OP.is_equal)
        Me = sing.tile([M, E], f32)
        nc.vector.tensor_scalar(out=Me[:], in0=S(0, M, C_IOE, C_IOE + E),
                                scalar1=S(0, M, C_ME, C_ME + 1), scalar2=None, op0=OP.is_equal)
        MeT = sing.tile([E, M], f32)
        nc.vector.tensor_scalar(out=MeT[:], in0=S(0, E, C_MERR, C_MERR + M),
                                scalar1=S(0, E, C_IOP96, C_IOP96 + 1), scalar2=None, op0=OP.is_equal)
        MqTa = sing.tile([P, M], f32)
        nc.vector.tensor_scalar(out=MqTa[:], in0=S(0, P, C_MQRA, C_MQRA + M),
                                scalar1=S(0, P, C_IOPA, C_IOPA + 1), scalar2=None, op0=OP.is_equal)
        MqTb = sing.tile([Q - P, M], f32)
        nc.vector.tensor_scalar(out=MqTb[:], in0=S(0, Q - P, C_MQRB, C_MQRB + M),
                                scalar1=S(0, Q - P, C_IOPB, C_IOPB + 1), scalar2=None, op0=OP.is_equal)

        # ---------- tiny matmul gathers ----------
        pts_ps = ps.tile([M, 2], f32)
        nc.tensor.matmul(out=pts_ps[:], lhsT=MeT[:], rhs=S(0, E, C_INC, C_INC + 2),
                         start=True, stop=True)
        ptsr = sing.tile([M, 2], f32)
        nc.vector.tensor_copy(out=ptsr[:], in_=pts_ps[:])

        # ---------- window offsets (channel-major: runs of 7 floats) ----------
        rmag = sing.tile([M, 2], f32)
        nc.vector.tensor_scalar(out=rmag[:], in0=ptsr[:], scalar1=MAGIC, scalar2=-MAGIC,
                                op0=OP.add, op1=OP.add)
        gtm = sing.tile([M, 2], f32)
        nc.vector.tensor_tensor(out=gtm[:], in0=rmag[:], in1=ptsr[:], op=OP.is_gt)
        pixf = sing.tile([M, 2], f32)
        nc.vector.tensor_tensor(out=pixf[:], in0=rmag[:], in1=gtm[:], op=OP.subtract)
        base = sing.tile([M, 1], f32)
        nc.vector.tensor_scalar(out=base[:], in0=pixf[:, 0:1], scalar1=float(W),
                                scalar2=float(-3 * W - 3), op0=OP.mult, op1=OP.add)
        nc.vector.tensor_tensor(out=base[:], in0=base[:], in1=pixf[:, 1:2], op=OP.add)
        sofs = sing.tile([M, WIN], f32)
        nc.vector.tensor_scalar(out=sofs[:], in0=S(0, M, C_DROF, C_DROF + WIN),
                                scalar1=base[:], scalar2=S(0, M, C_RB, C_RB + 1),
                                op0=OP.add, op1=OP.add)
        v1 = sing.tile([M, WIN], f32)
        nc.vector.tensor_scalar(out=v1[:], in0=sofs[:], scalar1=0.0, scalar2=None, op0=OP.is_ge)
        v2 = sing.tile([M, WIN], f32)
        nc.vector.tensor_scalar(out=v2[:], in0=sofs[:], scalar1=float(NPIX - 1), scalar2=None, op0=OP.is_le)
        valid = sing.tile([M, WIN], f32)
        nc.vector.tensor_tensor(out=valid[:], in0=v1[:], in1=v2[:], op=OP.mult)
        # one offset per m: start of the contiguous RUNW-float span. Clamped so
        # the span stays inside the padded flat tensor; straddle-up/down
        # windows are never clamped (border margin), so valid rows stay
        # row-aligned at run[a*W + b].
        clam0 = sing.tile([M, 1], f32)
        nc.vector.tensor_scalar(out=clam0[:], in0=sofs[:, 0:1], scalar1=float(-(6 * W + 4)),
                                scalar2=float(NPIX - WIN), op0=OP.max, op1=OP.min)
        soft = sing.tile([M, 1], f32)
        nc.vector.tensor_scalar(out=soft[:], in0=clam0[:], scalar1=S(0, M, C_MENP, C_MENP + 1),
                                scalar2=MAGIC + PADF, op0=OP.add, op1=OP.add)
        soft_i = sing.tile([M, 1], i32)
        nc.vector.tensor_scalar(out=soft_i[:], in0=soft[:].bitcast(i32), scalar1=0x007FFFFF,
                                scalar2=None, op0=OP.bitwise_and)
        sofb = sing.tile([M, 1], f32)
        nc.vector.tensor_scalar(out=sofb[:], in0=clam0[:], scalar1=S(0, M, C_MQNP, C_MQNP + 1),
                                scalar2=MAGIC + PADF, op0=OP.add, op1=OP.add)
        sofb_i = sing.tile([M, 1], i32)
        nc.vector.tensor_scalar(out=sofb_i[:], in0=sofb[:].bitcast(i32), scalar1=0x007FFFFF,
                                scalar2=None, op0=OP.bitwise_and)

        # ---------- window gathers: one RUNW-float run per matched electron ----------
        tw = sing.tile([M, RUNW], f32)
        bw = sing.tile([M, RUNW], f32)
        true_flat = bass.AP(tensor=true_cm.ap().tensor, offset=0,
                            ap=[[1, PADF + E * NPIX + PADB], [1, 1]])
        bin_flat = bass.AP(tensor=bin_cm.ap().tensor, offset=0,
                           ap=[[1, PADF + Q * NPIX + PADB], [1, 1]])
        if "win" in DIS:
            nc.vector.memset(tw[:], 0.0)
            nc.vector.memset(bw[:], 0.0)
        else:
            nc.gpsimd.indirect_dma_start(
                out=tw[:], out_offset=None, in_=true_flat,
                in_offset=bass.IndirectOffsetOnAxis(ap=soft_i[:], axis=0))
            nc.gpsimd.indirect_dma_start(
                out=bw[:], out_offset=None, in_=bin_flat,
                in_offset=bass.IndirectOffsetOnAxis(ap=sofb_i[:], axis=0))

        # ---------- remaining tiny matmul gathers ----------
        cen_ps = ps.tile([M, 2], f32)
        nc.tensor.matmul(out=cen_ps[:], lhsT=MqTa[:], rhs=S(0, P, C_POSA, C_POSA + 2),
                         start=True, stop=False)
        nc.tensor.matmul(out=cen_ps[:], lhsT=MqTb[:], rhs=S(0, Q - P, C_POSB, C_POSB + 2),
                         start=False, stop=True)
        cenr = sing.tile([M, 2], f32)
        nc.vector.tensor_copy(out=cenr[:], in_=cen_ps[:])

        chr_ps = ps.tile([M, 4], f32)
        nc.tensor.matmul(out=chr_ps[:], lhsT=MqTa[:], rhs=S(0, P, C_CHOLA, C_CHOLA + 4),
                         start=True, stop=False)
        nc.tensor.matmul(out=chr_ps[:], lhsT=MqTb[:], rhs=S(0, Q - P, C_CHOLB, C_CHOLB + 4),
                         start=False, stop=True)
        cholr = sing.tile([M, 4], f32)
        nc.vector.tensor_copy(out=cholr[:], in_=chr_ps[:])

        H_ps = ps.tile([E, Q], f32)
        nc.tensor.matmul(out=H_ps[:], lhsT=Me[:], rhs=Mq[:], start=True, stop=True)
        Hs = sing.tile([E, Q], f32)
        nc.vector.tensor_copy(out=Hs[:], in_=H_ps[:])

        # matched-q indicator, replicated to all partitions (column sums of Mq)
        ind_ps = ps.tile([P, Q], f32)
        nc.tensor.matmul(out=ind_ps[:], lhsT=onesw[:], rhs=Mq[:], start=True, stop=True)
        ind_bf = sing.tile([P, Q], bf16)
        nc.vector.tensor_copy(out=ind_bf[:], in_=ind_ps[:])
        ind1 = sing.tile([1, Q], f32)
        nc.vector.tensor_copy(out=ind1[:], in_=ind_ps[0:1, :])

        # ---------- dice streaming ----------
        from concourse.tile import add_dep_helper
        C_ps = ps.tile([E, Q], f32)
        HQ = Q // 2
        for c in range(NCHUNK):
            pk_t = pk_fifo.pop(0)
            por_v = pk_t[:, :, 0:Q]
            tru_v = pk_t[:, :, Q:CH]
            exp_t = big.tile([P, JC, Q], bf16, tag="exp")
            exp_i = nc.scalar.activation(out=exp_t[:], in_=por_v, func=AF.Exp)
            mexp_t = big.tile([P, JC, Q], bf16, tag="mexp")
            zq_t = big.tile([P, JC], f32, tag="zq")
            nc.vector.tensor_tensor(out=mexp_t[:], in0=exp_t[:], in1=bc(ind_bf[:], 1, JC), op=OP.mult)
            if "fold" in DIS:
                nc.vector.reduce_sum(out=zq_t[:], in_=mexp_t[:], axis=AX.X)
            else:
                # gpsimd folds the q-halves so DVE only reduces half the width
                fold_t = big.tile([P, JC, HQ], bf16, tag="fold")
                with nc.allow_low_precision(reason="bf16 partial sums feed an f32 reduce"):
                    nc.gpsimd.tensor_tensor(out=fold_t[:], in0=mexp_t[:, :, 0:HQ],
                                            in1=mexp_t[:, :, HQ:Q], op=OP.add)
                nc.vector.reduce_sum(out=zq_t[:], in_=fold_t[:], axis=AX.X)
            rz_t = big.tile([P, JC], bf16, tag="rz")
            with nc.allow_low_precision(reason="rz scales both num and den; error cancels in dice ratio"):
                nc.vector.reciprocal(out=rz_t[:], in_=zq_t[:])
            a_t = big.tile([P, JC, E], bf16, tag="a")
            nc.vector.tensor_tensor(out=a_t[:], in0=tru_v, in1=bc(rz_t[:], 2, E), op=OP.mult)
            for kb in range(JC):
                nc.tensor.matmul(out=C_ps[:], lhsT=a_t[:, kb, :], rhs=mexp_t[:, kb, :],
                                 start=(c == 0 and kb == 0),
                                 stop=(c == NCHUNK - 1 and kb == JC - 1))
            if c + 2 < NCHUNK:
                pk_fifo.append(issue_chunk(c + 2, dep=exp_i))

            # ---- work slotted into DVE slack between chunks ----
            if c == 2:
                # occupancy CE: lse part
                e4 = sing.tile([P, J, K], f32)
                nc.scalar.activation(out=e4[:], in_=occ_t[:, :, 0:K], func=AF.Exp)
                s4 = sing.tile([P, J], f32)
                nc.vector.reduce_sum(out=s4[:], in_=e4[:], axis=AX.X)
                lse = sing.tile([P, J], f32)
                emit_ln("occ", lse[:], s4[:], P, J)
            if c == 3:
                # occupancy CE: label select + partial sum
                xt = sing.tile([P, J], f32)
                mk = sing.tile([P, J], f32)
                pk2 = sing.tile([P, J], f32)
                for k in range(K):
                    nc.vector.tensor_scalar(out=mk[:], in0=occ_t[:, :, K], scalar1=float(k),
                                            scalar2=None, op0=OP.is_equal)
                    if k == 0:
                        nc.vector.tensor_tensor(out=xt[:], in0=mk[:], in1=occ_t[:, :, 0], op=OP.mult)
                    else:
                        nc.vector.tensor_tensor(out=pk2[:], in0=mk[:], in1=occ_t[:, :, k], op=OP.mult)
                        nc.vector.tensor_tensor(out=xt[:], in0=xt[:], in1=pk2[:], op=OP.add)
                nc.vector.tensor_tensor(out=lse[:], in0=lse[:], in1=xt[:], op=OP.subtract)
                nc.vector.reduce_sum(out=stats[:, 4:5], in_=lse[:], axis=AX.X)
            if c == 4:
                # 7x7 window BCE (gathers long since landed): the 49 window
                # values sit at run[a*W + b] — express as a strided AP
                def win_ap(t):
                    pdim = t[:].ap[0]
                    return bass.AP(tensor=t[:].tensor, offset=t[:].offset,
                                   ap=[pdim, [W, WIN], [1, WIN]])

                tv = sing.tile([M, WIN * WIN], f32)
                nc.vector.tensor_copy(out=tv[:].rearrange("m (a b) -> m a b", a=WIN),
                                      in_=win_ap(tw))
                lg = sing.tile([M, WIN * WIN], f32)
                nc.vector.tensor_copy(out=lg[:].rearrange("m (a b) -> m a b", a=WIN),
                                      in_=win_ap(bw))
                spw = sing.tile([M, WIN * WIN], f32)
                emit_softplus("win", spw[:], lg[:], M, WIN * WIN)
                prw = sing.tile([M, WIN * WIN], f32)
                nc.vector.tensor_tensor(out=prw[:], in0=lg[:], in1=tv[:], op=OP.mult)
                nc.vector.tensor_tensor(out=spw[:], in0=spw[:], in1=prw[:], op=OP.subtract)
                scr_w = sing.tile([M, WIN * WIN], f32)
                valid49 = sing.tile([M, WIN * WIN], f32)
                nc.vector.tensor_copy(out=valid49[:].rearrange("m (a b) -> m a b", a=WIN),
                                      in_=bc(valid[:], 2, WIN))
                nc.vector.tensor_tensor(out=scr_w[:], in0=spw[:], in1=valid49[:], op=OP.mult)
                nc.vector.reduce_sum(out=stats[0:M, 1:2], in_=scr_w[:], axis=AX.X)
            if c == 5:
                # class loss (partition 0)
                iel = S(0, 1, C_IEL, C_IEL + Q)
                sp = sing.tile([1, Q], f32)
                emit_softplus("cls", sp[:], iel, 1, Q)
                t9 = sing.tile([1, Q], f32)
                nc.vector.tensor_scalar(out=t9[:], in0=sp[:], scalar1=0.9, scalar2=None, op0=OP.mult)
                nc.vector.tensor_tensor(out=t9[:], in0=t9[:], in1=iel, op=OP.subtract)
                scr_q = sing.tile([1, Q], f32)
                clsm = sing.tile([1, 1], f32)
                nc.vector.tensor_tensor(out=scr_q[:], in0=t9[:], in1=ind1[:], op=OP.mult)
                nc.vector.reduce_sum(out=clsm[:], in_=scr_q[:], axis=AX.X)
                spsum = sing.tile([1, 1], f32)
                nc.vector.reduce_sum(out=spsum[:], in_=sp[:], axis=AX.X)
                nc.vector.tensor_scalar(out=spsum[:], in0=spsum[:], scalar1=NO_E, scalar2=None, op0=OP.mult)
                nc.vector.tensor_tensor(out=res[:, 6:7], in0=spsum[:], in1=clsm[:], op=OP.add)
            if c == 6:
                # NLL (96 partitions)
                d_ = sing.tile([M, 2], f32)
                nc.vector.tensor_tensor(out=d_[:], in0=ptsr[:], in1=cenr[:], op=OP.subtract)
                r00 = sing.tile([M, 1], f32)
                nc.vector.reciprocal(out=r00[:], in_=cholr[:, 0:1])
                r11 = sing.tile([M, 1], f32)
                nc.vector.reciprocal(out=r11[:], in_=cholr[:, 3:4])
                z0 = sing.tile([M, 1], f32)
                nc.vector.tensor_tensor(out=z0[:], in0=d_[:, 0:1], in1=r00[:], op=OP.mult)
                t1 = sing.tile([M, 1], f32)
                nc.vector.tensor_tensor(out=t1[:], in0=cholr[:, 2:3], in1=z0[:], op=OP.mult)
                nc.vector.tensor_tensor(out=t1[:], in0=d_[:, 1:2], in1=t1[:], op=OP.subtract)
                z1 = sing.tile([M, 1], f32)
                nc.vector.tensor_tensor(out=z1[:], in0=t1[:], in1=r11[:], op=OP.mult)
                sq = sing.tile([M, 1], f32)
                nc.vector.tensor_tensor(out=sq[:], in0=z0[:], in1=z0[:], op=OP.mult)
                sq1 = sing.tile([M, 1], f32)
                nc.vector.tensor_tensor(out=sq1[:], in0=z1[:], in1=z1[:], op=OP.mult)
                nc.vector.tensor_tensor(out=sq[:], in0=sq[:], in1=sq1[:], op=OP.add)
                ldet = sing.tile([M, 1], f32)
                nc.vector.tensor_tensor(out=ldet[:], in0=cholr[:, 0:1], in1=cholr[:, 3:4], op=OP.mult)
                lnd = sing.tile([M, 1], f32)
                emit_ln("nld", lnd[:], ldet[:], M, 1)
                nc.vector.tensor_scalar(out=sq[:], in0=sq[:], scalar1=0.5,
                                        scalar2=float(np.log(2.0 * np.pi)), op0=OP.mult, op1=OP.add)
                nc.vector.tensor_tensor(out=stats[0:M, 0:1], in0=sq[:], in1=lnd[:], op=OP.add)

        # ---------- dice epilogue ----------
        Cs = sing.tile([E, Q], f32)
        nc.vector.tensor_copy(out=Cs[:], in_=C_ps[:])
        # C's rhs was masked exp, so sum_q C[e,q] = sum_pixels true[p,e]: den free.
        nc.vector.reduce_sum(out=stats[0:E, 3:4], in_=Cs[:], axis=AX.X)
        scr_c = sing.tile([E, Q], f32)
        nc.vector.tensor_tensor(out=scr_c[:], in0=Cs[:], in1=Hs[:], op=OP.mult)
        nc.vector.reduce_sum(out=stats[0:M, 2:3], in_=scr_c[:], axis=AX.X)

        # ---------- final cross-partition reduction ----------
        fin_ps = ps.tile([1, 6], f32)
        nc.tensor.matmul(out=fin_ps[:], lhsT=ones[:], rhs=stats[:], start=True, stop=True)
        nc.vector.tensor_copy(out=res[:, 0:6], in_=fin_ps[:])
        nc.sync.dma_start(out=partials.ap(), in_=res[:])

    nc.compile()
    return nc


def _get_nc():
    if "nc" not in _CACHE:
        _CACHE["nc"] = _build_nc()
    return _CACHE["nc"]


def make_in_maps(is_electron_logit, true_segmap, binary_mask_logits, portion_logits,
                 incidence_points, positions, chol, occupancy_logits, occupancy_true,
                 matched_q, matched_e):
    f = np.float32
    in_maps = []
    for c in range(8):
        b, h = c // 2, c % 2
        sl = slice(h * HALF, (h + 1) * HALF)
        me = np.asarray(matched_e[b]).astype(f)
        mq = np.asarray(matched_q[b]).astype(f)
        chol_b = np.asarray(chol[b], dtype=f).reshape(Q, 4)
        pos_b = np.asarray(positions[b], dtype=f)
        true_sl = np.ascontiguousarray(true_segmap[b, sl], dtype=f).reshape(NPIX, E)
        por_sl = np.ascontiguousarray(portion_logits[b, sl], dtype=f).reshape(NPIX, Q)
        bin_sl = np.ascontiguousarray(binary_mask_logits[b, sl], dtype=f).reshape(NPIX, Q)
        stream = np.concatenate([por_sl, true_sl], axis=1).reshape(NCHUNK * P, JC, CH)
        occ_sl = np.asarray(occupancy_logits[b, sl], dtype=f).reshape(P, J, K)
        occt = np.asarray(occupancy_true[b, sl], dtype=f).reshape(P, J, 1)
        occ_pack = np.concatenate([occ_sl, occt], axis=2)

        sm = np.zeros((P, SC), dtype=f)
        sm[:M, C_ME] = me
        sm[:M, C_MQ] = mq
        sm[:M, C_RB] = -h * NPIX
        sm[:M, C_INC:C_INC + 2] = np.asarray(incidence_points[b], dtype=f)
        sm[:M, C_DROF:C_DROF + WIN] = np.tile(np.arange(WIN, dtype=f) * W, (M, 1))
        sm[:E, C_IOP96] = np.arange(E, dtype=f)
        sm[:P, C_IOPA] = np.arange(P, dtype=f)
        sm[:Q - P, C_IOPB] = np.arange(P, Q, dtype=f)
        sm[:P, C_POSA:C_POSA + 2] = pos_b[:P]
        sm[:Q - P, C_POSB:C_POSB + 2] = pos_b[P:]
        sm[:P, C_CHOLA:C_CHOLA + 4] = chol_b[:P]
        sm[:Q - P, C_CHOLB:C_CHOLB + 4] = chol_b[P:]
        sm[:E, C_MERR:C_MERR + M] = np.tile(me, (E, 1))
        sm[:P, C_MQRA:C_MQRA + M] = np.tile(mq, (P, 1))
        sm[:Q - P, C_MQRB:C_MQRB + M] = np.tile(mq, (Q - P, 1))
        sm[:M, C_IOQ:C_IOQ + Q] = np.tile(np.arange(Q, dtype=f), (M, 1))
        sm[:M, C_IOE:C_IOE + E] = np.tile(np.arange(E, dtype=f), (M, 1))
        sm[0, C_IEL:C_IEL + Q] = np.asarray(is_electron_logit, dtype=f).reshape(B, Q)[b]
        sm[:M, C_MENP] = me * NPIX
        sm[:M, C_MQNP] = mq * NPIX

        def flat_pad(cm):
            out = np.zeros((1, PADF + cm.size + PADB), dtype=f)
            out[0, PADF:PADF + cm.size] = cm.reshape(-1)
            return out

        in_maps.append(dict(
            stream=stream,
            occ_pack=occ_pack,
            true_cm=flat_pad(np.ascontiguousarray(true_sl.T)),
            bin_cm=flat_pad(np.ascontiguousarray(bin_sl.T)),
            smalls=sm,
        ))
    return in_maps


def combine(partials_list):
    s = np.stack([np.asarray(p, dtype=np.float64).reshape(8) for p in partials_list])
    # slots: 0=nll_sum 1=bce_sum 2=num2_sum 3=den_true_sum 4=occ_sum 6=class_sum
    class_loss = s[0::2, 6].sum() / (B * Q)
    nll_loss = s[0::2, 0].sum() / (B * M)
    bce_loss = s[:, 1].sum() / (B * M * WIN * WIN)
    occ_loss = s[:, 4].sum() / (B * H * W)
    dice = 0.0
    for b in range(B):
        num = 2.0 * (s[2 * b, 2] + s[2 * b + 1, 2])
        den = s[2 * b, 3] + s[2 * b + 1, 3] + H * W
        dice += 1.0 - (num + 1.0) / (den + 1.0)
    dice_loss = dice / B
    return np.float32(class_loss + bce_loss + dice_loss + nll_loss + occ_loss)


def kernel(**inputs):
    from concourse.bass_utils import run_bass_kernel_spmd
    nc = _get_nc()
    in_maps = make_in_maps(**{k: np.asarray(v) for k, v in inputs.items()})
    r = run_bass_kernel_spmd(nc, in_maps, list(range(8)))
    return combine([r.results[c]["partials"] for c in range(8)])


# revision 24
# speedup vs baseline: 1.5342x; 1.5342x over previous
"""Trainium2 Bass kernel for nn_Criterion_32830730011569.

8 cores = (image b in 0..3) x (H-half h in 0..1). Host-side prep is pure
indexing/layout (slice, transpose, channel-gather by matched_q/matched_e,
pack) — all arithmetic runs on device.

Each core streams its [96,192] pixel slice once as a packed
[NCHUNK*P, JC, 192] tensor (matched-por || matched-true, chunk-contiguous
1.42MB linear DMAs, staggered prefetch):
  - dice: exp on ACT; softmax denominator is one DVE reduce (channels are
    pre-gathered so no mask is needed); bf16 matmuls accumulate
    C[m1,m2] = (true_r/Z)^T exp_r into PSUM. num = 2*trace(C); den =
    sum of all C entries (each row of exp_r/Z sums to 1).
  - 7x7-window BCE: true/bin shipped channel-major + padded so each m's
    whole window lives in one contiguous 1159-float run; ONE indirect DMA
    per tensor gathers all 96 runs; BCE uses ACT Softplus.
  - occupancy CE: streamed exp + ACT Ln logsumexp + label select.
  - class / NLL: tiny per-query math from host-gathered rows, ACT Ln.
Small setup inputs ride in one packed [128,SC] tensor on the Scalar
engine's HWDGE queue. Each core returns 7 partial sums; the host combines.
"""
import sys

sys.path.insert(0, "/opt/trn_rl_repo")
import numpy as np

B, H, W, Q, E, M, K, WIN = 4, 192, 192, 160, 96, 96, 4, 7
NO_E = 0.1
HALF = H // 2          # rows per core slice
NPIX = HALF * W        # 18432 pixels per slice
P = 128                # partitions
J = NPIX // P          # 144 pixels per partition
NCHUNK = 8
JC = J // NCHUNK       # 18
CH = 2 * M             # 192 packed channels (por_r | true_r)
MAGIC = 8388608.0      # 2^23
MAGIC_I = 0x4B000000
RUNW = 6 * W + 7       # one contiguous span covering a whole 7x7 window
PADF = RUNW            # front pad so straddle-up windows stay row-aligned
PADB = 2400

# smalls column map
C_RB, C_INC, C_DROF, C_POS, C_CHOL, C_MENP, C_MQNP = 0, 1, 3, 10, 12, 16, 17
C_IEL, C_IND, C_I96 = 18, 178, 338
SC = 338 + M

_CACHE = {}


def _build_nc():
    import concourse.bass as bass
    import concourse.bacc as bacc
    import concourse.tile as tile
    from concourse import mybir
    from concourse.tile import add_dep_helper

    f32 = mybir.dt.float32
    i32 = mybir.dt.int32
    bf16 = mybir.dt.bfloat16
    AF = mybir.ActivationFunctionType
    OP = mybir.AluOpType
    AX = mybir.AxisListType

    nc = bacc.Bacc("TRN2", target_bir_lowering=False, debug=False, num_devices=8)

    # ---- external I/O ----
    stream = nc.dram_tensor("stream", [NCHUNK * P, JC, CH], f32, kind="ExternalInput")
    occ_pack = nc.dram_tensor("occ_pack", [P, J, K + 1], f32, kind="ExternalInput")
    true_cm = nc.dram_tensor("true_cm", [1, PADF + E * NPIX + PADB], f32, kind="ExternalInput")
    bin_cm = nc.dram_tensor("bin_cm", [1, PADF + Q * NPIX + PADB], f32, kind="ExternalInput")
    smalls = nc.dram_tensor("smalls", [P, SC], f32, kind="ExternalInput")
    partials = nc.dram_tensor("partials", [1, 8], f32, kind="ExternalOutput")

    def bc(ap, pos, count):
        """Insert a stride-0 broadcast dim into an AP at free-dim position pos."""
        new = list(ap.ap)
        new.insert(pos, [0, count])
        return bass.AP(tensor=ap.tensor, offset=ap.offset, ap=new)

    from contextlib import ExitStack

    with tile.TileContext(nc) as tc, ExitStack() as ctx:
        sing = ctx.enter_context(tc.tile_pool(name="sing", bufs=1))
        pkp = ctx.enter_context(tc.tile_pool(name="pkp", bufs=3))
        big = ctx.enter_context(tc.tile_pool(name="big", bufs=2))
        ps = ctx.enter_context(tc.tile_pool(name="ps", bufs=1, space="PSUM"))

        # ---------- DMA issues: chunk stream on Sync, smalls/occ on Scalar ----------
        def issue_chunk(c, dep=None):
            t = pkp.tile([P, JC, CH], f32, tag="pk")
            d = nc.sync.dma_start(out=t[:], in_=stream.ap()[c * P:(c + 1) * P, :, :])
            if dep is not None:
                # stagger: keep at most ~2 chunk transfers interleaving on the
                # queue so the earliest chunk completes soonest
                add_dep_helper(d.ins, dep.ins, reason="stagger prefetch")
            return t

        sm = sing.tile([P, SC], f32)
        nc.scalar.dma_start(out=sm[:], in_=smalls.ap())
        occ_t = sing.tile([P, J, K + 1], f32)
        nc.scalar.dma_start(out=occ_t[:], in_=occ_pack.ap())
        pk_fifo = [issue_chunk(0), issue_chunk(1)]

        def S(p0, p1, c0, c1):
            return sm[p0:p1, c0:c1]

        ones = sing.tile([P, 1], f32)
        nc.vector.memset(ones[:], 1.0)
        stats = sing.tile([P, 6], f32)
        nc.vector.memset(stats[:], 0.0)
        res = sing.tile([1, 8], f32)
        nc.vector.memset(res[:], 0.0)

        # ---------- window offsets (first DVE work; needs only smalls) ----------
        ptsr = S(0, M, C_INC, C_INC + 2)
        rmag = sing.tile([M, 2], f32)
        nc.vector.tensor_scalar(out=rmag[:], in0=ptsr, scalar1=MAGIC, scalar2=-MAGIC,
                                op0=OP.add, op1=OP.add)
        gtm = sing.tile([M, 2], f32)
        nc.vector.tensor_tensor(out=gtm[:], in0=rmag[:], in1=ptsr, op=OP.is_gt)
        pixf = sing.tile([M, 2], f32)
        nc.vector.tensor_tensor(out=pixf[:], in0=rmag[:], in1=gtm[:], op=OP.subtract)
        base = sing.tile([M, 1], f32)
        nc.vector.tensor_scalar(out=base[:], in0=pixf[:, 0:1], scalar1=float(W),
                                scalar2=float(-3 * W - 3), op0=OP.mult, op1=OP.add)
        nc.vector.tensor_tensor(out=base[:], in0=base[:], in1=pixf[:, 1:2], op=OP.add)
        sofs = sing.tile([M, WIN], f32)
        nc.vector.tensor_scalar(out=sofs[:], in0=S(0, M, C_DROF, C_DROF + WIN),
                                scalar1=base[:], scalar2=S(0, M, C_RB, C_RB + 1),
                                op0=OP.add, op1=OP.add)
        v1 = sing.tile([M, WIN], f32)
        nc.vector.tensor_scalar(out=v1[:], in0=sofs[:], scalar1=0.0, scalar2=None, op0=OP.is_ge)
        v2 = sing.tile([M, WIN], f32)
        nc.vector.tensor_scalar(out=v2[:], in0=sofs[:], scalar1=float(NPIX - 1), scalar2=None, op0=OP.is_le)
        valid = sing.tile([M, WIN], f32)
        nc.vector.tensor_tensor(out=valid[:], in0=v1[:], in1=v2[:], op=OP.mult)
        # one offset per m: start of the contiguous RUNW-float span, clamped so
        # the span stays inside the padded flat tensor. Border margins mean
        # straddling windows are never clamped, so valid rows stay row-aligned.
        clam0 = sing.tile([M, 1], f32)
        nc.vector.tensor_scalar(out=clam0[:], in0=sofs[:, 0:1], scalar1=float(-(6 * W + 4)),
                                scalar2=float(NPIX - WIN), op0=OP.max, op1=OP.min)
        soft = sing.tile([M, 1], f32)
        nc.vector.tensor_scalar(out=soft[:], in0=clam0[:], scalar1=S(0, M, C_MENP, C_MENP + 1),
                                scalar2=MAGIC + PADF, op0=OP.add, op1=OP.add)
        soft_i = sing.tile([M, 1], i32)
        nc.vector.tensor_scalar(out=soft_i[:], in0=soft[:].bitcast(i32), scalar1=0x007FFFFF,
                                scalar2=None, op0=OP.bitwise_and)
        sofb = sing.tile([M, 1], f32)
        nc.vector.tensor_scalar(out=sofb[:], in0=clam0[:], scalar1=S(0, M, C_MQNP, C_MQNP + 1),
                                scalar2=MAGIC + PADF, op0=OP.add, op1=OP.add)
        sofb_i = sing.tile([M, 1], i32)
        nc.vector.tensor_scalar(out=sofb_i[:], in0=sofb[:].bitcast(i32), scalar1=0x007FFFFF,
                                scalar2=None, op0=OP.bitwise_and)

        # ---------- window gathers: one RUNW-float run per matched electron ----------
        tw = sing.tile([M, RUNW], f32)
        bw = sing.tile([M, RUNW], f32)
        true_flat = bass.AP(tensor=true_cm.ap().tensor, offset=0,
                            ap=[[1, PADF + E * NPIX + PADB], [1, 1]])
        bin_flat = bass.AP(tensor=bin_cm.ap().tensor, offset=0,
                           ap=[[1, PADF + Q * NPIX + PADB], [1, 1]])
        nc.gpsimd.indirect_dma_start(
            out=tw[:], out_offset=None, in_=true_flat,
            in_offset=bass.IndirectOffsetOnAxis(ap=soft_i[:], axis=0))
        nc.gpsimd.indirect_dma_start(
            out=bw[:], out_offset=None, in_=bin_flat,
            in_offset=bass.IndirectOffsetOnAxis(ap=sofb_i[:], axis=0))

        # ---------- dice streaming ----------
        C_ps = ps.tile([M, M], f32)
        for c in range(NCHUNK):
            pk_t = pk_fifo.pop(0)
            por_v = pk_t[:, :, 0:M]
            tru_v = pk_t[:, :, M:CH]
            exp_t = big.tile([P, JC, M], bf16, tag="exp")
            exp_i = nc.scalar.activation(out=exp_t[:], in_=por_v, func=AF.Exp)
            zq_t = big.tile([P, JC], f32, tag="zq")
            nc.vector.reduce_sum(out=zq_t[:], in_=exp_t[:], axis=AX.X)
            rz_t = big.tile([P, JC], bf16, tag="rz")
            with nc.allow_low_precision(reason="rz scales both num and den; error cancels in dice ratio"):
                nc.vector.reciprocal(out=rz_t[:], in_=zq_t[:])
            a_t = big.tile([P, JC, M], bf16, tag="a")
            nc.vector.tensor_tensor(out=a_t[:], in0=tru_v, in1=bc(rz_t[:], 2, M), op=OP.mult)
            for kb in range(JC):
                nc.tensor.matmul(out=C_ps[:], lhsT=a_t[:, kb, :], rhs=exp_t[:, kb, :],
                                 start=(c == 0 and kb == 0),
                                 stop=(c == NCHUNK - 1 and kb == JC - 1))
            if c + 2 < NCHUNK:
                pk_fifo.append(issue_chunk(c + 2, dep=exp_i))

            # ---- work slotted into engine slack between chunks ----
            if c == 2:
                # occupancy CE: logsumexp part (Ln on ACT)
                e4 = sing.tile([P, J, K], f32)
                nc.scalar.activation(out=e4[:], in_=occ_t[:, :, 0:K], func=AF.Exp)
                s4 = sing.tile([P, J], f32)
                nc.vector.reduce_sum(out=s4[:], in_=e4[:], axis=AX.X)
                lse = sing.tile([P, J], f32)
                nc.scalar.activation(out=lse[:], in_=s4[:], func=AF.Ln)
            if c == 3:
                # occupancy CE: label select + partial sum
                xt = sing.tile([P, J], f32)
                mk = sing.tile([P, J], f32)
                pk2 = sing.tile([P, J], f32)
                for k in range(K):
                    nc.vector.tensor_scalar(out=mk[:], in0=occ_t[:, :, K], scalar1=float(k),
                                            scalar2=None, op0=OP.is_equal)
                    if k == 0:
                        nc.vector.tensor_tensor(out=xt[:], in0=mk[:], in1=occ_t[:, :, 0], op=OP.mult)
                    else:
                        nc.vector.tensor_tensor(out=pk2[:], in0=mk[:], in1=occ_t[:, :, k], op=OP.mult)
                        nc.vector.tensor_tensor(out=xt[:], in0=xt[:], in1=pk2[:], op=OP.add)
                nc.vector.tensor_tensor(out=lse[:], in0=lse[:], in1=xt[:], op=OP.subtract)
                nc.vector.reduce_sum(out=stats[:, 4:5], in_=lse[:], axis=AX.X)
            if c == 4:
                # 7x7 window BCE: the 49 window values sit at run[a*W + b]
                def win_ap(t):
                    pdim = t[:].ap[0]
                    return bass.AP(tensor=t[:].tensor, offset=t[:].offset,
                                   ap=[pdim, [W, WIN], [1, WIN]])

                tv = sing.tile([M, WIN * WIN], f32)
                nc.vector.tensor_copy(out=tv[:].rearrange("m (a b) -> m a b", a=WIN),
                                      in_=win_ap(tw))
                lg = sing.tile([M, WIN * WIN], f32)
                nc.vector.tensor_copy(out=lg[:].rearrange("m (a b) -> m a b", a=WIN),
                                      in_=win_ap(bw))
                # softplus = Ln(1 + exp): Exp/Ln share one ACT table
                exw = sing.tile([M, WIN * WIN], f32)
                nc.scalar.activation(out=exw[:], in_=lg[:], func=AF.Exp)
                nc.vector.tensor_scalar(out=exw[:], in0=exw[:], scalar1=1.0, scalar2=None, op0=OP.add)
                spw = sing.tile([M, WIN * WIN], f32)
                nc.scalar.activation(out=spw[:], in_=exw[:], func=AF.Ln)
            if c == 5:
                prw = sing.tile([M, WIN * WIN], f32)
                nc.vector.tensor_tensor(out=prw[:], in0=lg[:], in1=tv[:], op=OP.mult)
                nc.vector.tensor_tensor(out=spw[:], in0=spw[:], in1=prw[:], op=OP.subtract)
                scr_w = sing.tile([M, WIN * WIN], f32)
                valid49 = sing.tile([M, WIN * WIN], f32)
                nc.vector.tensor_copy(out=valid49[:].rearrange("m (a b) -> m a b", a=WIN),
                                      in_=bc(valid[:], 2, WIN))
                nc.vector.tensor_tensor(out=scr_w[:], in0=spw[:], in1=valid49[:], op=OP.mult)
                nc.vector.reduce_sum(out=stats[0:M, 1:2], in_=scr_w[:], axis=AX.X)
            if c == 6:
                # class loss (partition 0, Softplus on ACT)
                iel = S(0, 1, C_IEL, C_IEL + Q)
                ind1 = S(0, 1, C_IND, C_IND + Q)
                exc = sing.tile([1, Q], f32)
                nc.scalar.activation(out=exc[:], in_=iel, func=AF.Exp)
                nc.vector.tensor_scalar(out=exc[:], in0=exc[:], scalar1=1.0, scalar2=None, op0=OP.add)
                sp = sing.tile([1, Q], f32)
                nc.scalar.activation(out=sp[:], in_=exc[:], func=AF.Ln)
                t9 = sing.tile([1, Q], f32)
                nc.vector.tensor_scalar(out=t9[:], in0=sp[:], scalar1=0.9, scalar2=None, op0=OP.mult)
                nc.vector.tensor_tensor(out=t9[:], in0=t9[:], in1=iel, op=OP.subtract)
                scr_q = sing.tile([1, Q], f32)
                clsm = sing.tile([1, 1], f32)
                nc.vector.tensor_tensor(out=scr_q[:], in0=t9[:], in1=ind1, op=OP.mult)
                nc.vector.reduce_sum(out=clsm[:], in_=scr_q[:], axis=AX.X)
                spsum = sing.tile([1, 1], f32)
                nc.vector.reduce_sum(out=spsum[:], in_=sp[:], axis=AX.X)
                nc.vector.tensor_scalar(out=spsum[:], in0=spsum[:], scalar1=NO_E, scalar2=None, op0=OP.mult)
                nc.vector.tensor_tensor(out=res[:, 6:7], in0=spsum[:], in1=clsm[:], op=OP.add)
            if c == 7:
                # NLL (96 partitions; chol/centers host-gathered)
                cenr = S(0, M, C_POS, C_POS + 2)
                chol0 = S(0, M, C_CHOL, C_CHOL + 1)
                chol1 = S(0, M, C_CHOL + 2, C_CHOL + 3)
                chol3 = S(0, M, C_CHOL + 3, C_CHOL + 4)
                d_ = sing.tile([M, 2], f32)
                nc.vector.tensor_tensor(out=d_[:], in0=ptsr, in1=cenr, op=OP.subtract)
                r00 = sing.tile([M, 1], f32)
                nc.vector.reciprocal(out=r00[:], in_=chol0)
                r11 = sing.tile([M, 1], f32)
                nc.vector.reciprocal(out=r11[:], in_=chol3)
                z0 = sing.tile([M, 1], f32)
                nc.vector.tensor_tensor(out=z0[:], in0=d_[:, 0:1], in1=r00[:], op=OP.mult)
                t1 = sing.tile([M, 1], f32)
                nc.vector.tensor_tensor(out=t1[:], in0=chol1, in1=z0[:], op=OP.mult)
                nc.vector.tensor_tensor(out=t1[:], in0=d_[:, 1:2], in1=t1[:], op=OP.subtract)
                z1 = sing.tile([M, 1], f32)
                nc.vector.tensor_tensor(out=z1[:], in0=t1[:], in1=r11[:], op=OP.mult)
                sq = sing.tile([M, 1], f32)
                nc.vector.tensor_tensor(out=sq[:], in0=z0[:], in1=z0[:], op=OP.mult)
                sq1 = sing.tile([M, 1], f32)
                nc.vector.tensor_tensor(out=sq1[:], in0=z1[:], in1=z1[:], op=OP.mult)
                nc.vector.tensor_tensor(out=sq[:], in0=sq[:], in1=sq1[:], op=OP.add)
                ldet = sing.tile([M, 1], f32)
                nc.vector.tensor_tensor(out=ldet[:], in0=chol0, in1=chol3, op=OP.mult)
                lnd = sing.tile([M, 1], f32)
                nc.scalar.activation(out=lnd[:], in_=ldet[:], func=AF.Ln)
                nc.vector.tensor_scalar(out=sq[:], in0=sq[:], scalar1=0.5,
                                        scalar2=float(np.log(2.0 * np.pi)), op0=OP.mult, op1=OP.add)
                nc.vector.tensor_tensor(out=stats[0:M, 0:1], in0=sq[:], in1=lnd[:], op=OP.add)

        # ---------- dice epilogue ----------
        Cs = sing.tile([M, M], f32)
        nc.vector.tensor_copy(out=Cs[:], in_=C_ps[:])
        # rhs rows (exp_r/Z) sum to 1, so summing all of C gives sum(true): den.
        nc.vector.reduce_sum(out=stats[0:M, 3:4], in_=Cs[:], axis=AX.X)
        scr_c = sing.tile([M, M], f32)
        nc.vector.tensor_tensor(out=scr_c[:], in0=Cs[:], in1=S(0, M, C_I96, C_I96 + M), op=OP.mult)
        nc.vector.reduce_sum(out=stats[0:M, 2:3], in_=scr_c[:], axis=AX.X)

        # ---------- final cross-partition reduction ----------
        fin_ps = ps.tile([1, 6], f32)
        nc.tensor.matmul(out=fin_ps[:], lhsT=ones[:], rhs=stats[:], start=True, stop=True)
        nc.vector.tensor_copy(out=res[:, 0:6], in_=fin_ps[:])
        nc.sync.dma_start(out=partials.ap(), in_=res[:])

    nc.compile()
    return nc


def _get_nc():
    if "nc" not in _CACHE:
        _CACHE["nc"] = _build_nc()
    return _CACHE["nc"]


def make_in_maps(is_electron_logit, true_segmap, binary_mask_logits, portion_logits,
                 incidence_points, positions, chol, occupancy_logits, occupancy_true,
                 matched_q, matched_e):
    f = np.float32
    in_maps = []
    for c in range(8):
        b, h = c // 2, c % 2
        sl = slice(h * HALF, (h + 1) * HALF)
        me = np.asarray(matched_e[b])
        mq = np.asarray(matched_q[b])
        true_sl = np.ascontiguousarray(true_segmap[b, sl], dtype=f).reshape(NPIX, E)
        por_sl = np.ascontiguousarray(portion_logits[b, sl], dtype=f).reshape(NPIX, Q)
        bin_sl = np.ascontiguousarray(binary_mask_logits[b, sl], dtype=f).reshape(NPIX, Q)
        # channel gathers: pure indexing (reference's take_along_axis layout)
        stream = np.concatenate([por_sl[:, mq], true_sl[:, me]], axis=1)
        stream = np.ascontiguousarray(stream).reshape(NCHUNK * P, JC, CH)
        occ_sl = np.asarray(occupancy_logits[b, sl], dtype=f).reshape(P, J, K)
        occt = np.asarray(occupancy_true[b, sl], dtype=f).reshape(P, J, 1)
        occ_pack = np.concatenate([occ_sl, occt], axis=2)

        sm = np.zeros((P, SC), dtype=f)
        sm[:M, C_RB] = -h * NPIX
        sm[:M, C_INC:C_INC + 2] = np.asarray(incidence_points[b], dtype=f)[me]
        sm[:M, C_DROF:C_DROF + WIN] = np.tile(np.arange(WIN, dtype=f) * W, (M, 1))
        sm[:M, C_POS:C_POS + 2] = np.asarray(positions[b], dtype=f)[mq]
        sm[:M, C_CHOL:C_CHOL + 4] = np.asarray(chol[b], dtype=f).reshape(Q, 4)[mq]
        sm[:M, C_MENP] = me.astype(f) * NPIX
        sm[:M, C_MQNP] = mq.astype(f) * NPIX
        sm[0, C_IEL:C_IEL + Q] = np.asarray(is_electron_logit, dtype=f).reshape(B, Q)[b]
        ind = np.zeros(Q, dtype=f)
        ind[mq] = 1.0
        sm[0, C_IND:C_IND + Q] = ind
        sm[:M, C_I96:C_I96 + M] = np.eye(M, dtype=f)

        def flat_pad(cm):
            out = np.zeros((1, PADF + cm.size + PADB), dtype=f)
            out[0, PADF:PADF + cm.size] = cm.reshape(-1)
            return out

        in_maps.append(dict(
            stream=stream,
            occ_pack=occ_pack,
            true_cm=flat_pad(np.ascontiguousarray(true_sl.T)),
            bin_cm=flat_pad(np.ascontiguousarray(bin_sl.T)),
            smalls=sm,
        ))
    return in_maps


def combine(partials_list):
    s = np.stack([np.asarray(p, dtype=np.float64).reshape(8) for p in partials_list])
    # slots: 0=nll_sum 1=bce_sum 2=num2_sum 3=den_true_sum 4=occ_sum 6=class_sum
    class_loss = s[0::2, 6].sum() / (B * Q)
    nll_loss = s[0::2, 0].sum() / (B * M)
    bce_loss = s[:, 1].sum() / (B * M * WIN * WIN)
    occ_loss = s[:, 4].sum() / (B * H * W)
    dice = 0.0
    for b in range(B):
        num = 2.0 * (s[2 * b, 2] + s[2 * b + 1, 2])
        den = s[2 * b, 3] + s[2 * b + 1, 3] + H * W
        dice += 1.0 - (num + 1.0) / (den + 1.0)
    dice_loss = dice / B
    return np.float32(class_loss + bce_loss + dice_loss + nll_loss + occ_loss)


def kernel(**inputs):
    from concourse.bass_utils import run_bass_kernel_spmd
    nc = _get_nc()
    in_maps = make_in_maps(**{k: np.asarray(v) for k, v in inputs.items()})
    r = run_bass_kernel_spmd(nc, in_maps, list(range(8)))
    return combine([r.results[c]["partials"] for c in range(8)])


# revision 27
# speedup vs baseline: 1.6485x; 1.0745x over previous
"""Trainium2 Bass kernel for nn_Criterion_32830730011569.

8 cores = (image b in 0..3) x (H-half h in 0..1). Host-side prep is pure
indexing/layout (slice, transpose, channel-gather by matched_q/matched_e,
pack) — all arithmetic runs on device.

Each core streams its [96,192] pixel slice once as a packed
[NCHUNK*P, JC, 192] tensor (matched-por || matched-true, chunk-contiguous
1.42MB linear DMAs, staggered prefetch):
  - dice: exp on ACT; softmax denominator is one DVE reduce (channels are
    pre-gathered so no mask is needed); bf16 matmuls accumulate
    C[m1,m2] = (true_r/Z)^T exp_r into PSUM. num = 2*trace(C); den =
    sum of all C entries (each row of exp_r/Z sums to 1).
  - 7x7-window BCE: true/bin shipped channel-major + padded so each m's
    whole window lives in one contiguous 1159-float run; ONE indirect DMA
    per tensor gathers all 96 runs; BCE uses ACT Softplus.
  - occupancy CE: streamed exp + ACT Ln logsumexp + label select.
  - class / NLL: tiny per-query math from host-gathered rows, ACT Ln.
Small setup inputs ride in one packed [128,SC] tensor on the Scalar
engine's HWDGE queue. Each core returns 7 partial sums; the host combines.
"""
import sys

sys.path.insert(0, "/opt/trn_rl_repo")
import numpy as np

B, H, W, Q, E, M, K, WIN = 4, 192, 192, 160, 96, 96, 4, 7
NO_E = 0.1
HALF = H // 2          # rows per core slice
NPIX = HALF * W        # 18432 pixels per slice
P = 128                # partitions
J = NPIX // P          # 144 pixels per partition
NCHUNK = 8
JC = J // NCHUNK       # 18
CH = 2 * M             # 192 packed channels (por_r | true_r)
MAGIC = 8388608.0      # 2^23
MAGIC_I = 0x4B000000
RUNW = 6 * W + 7       # one contiguous span covering a whole 7x7 window
PADF = RUNW            # front pad so straddle-up windows stay row-aligned
PADB = 2400

# smalls column map
C_RB, C_INC, C_DROF, C_POS, C_CHOL, C_MENP, C_MQNP = 0, 1, 3, 10, 12, 16, 17
C_IEL, C_IND, C_I96 = 18, 178, 338
SC = 338 + M

_CACHE = {}


def _build_nc():
    import concourse.bass as bass
    import concourse.bacc as bacc
    import concourse.tile as tile
    from concourse import mybir
    from concourse.tile import add_dep_helper

    f32 = mybir.dt.float32
    i32 = mybir.dt.int32
    bf16 = mybir.dt.bfloat16
    AF = mybir.ActivationFunctionType
    OP = mybir.AluOpType
    AX = mybir.AxisListType

    nc = bacc.Bacc("TRN2", target_bir_lowering=False, debug=False, num_devices=8)

    # ---- external I/O ----
    stream = nc.dram_tensor("stream", [NCHUNK * P, JC, CH], f32, kind="ExternalInput")
    occ_pack = nc.dram_tensor("occ_pack", [P, J, K + 1], f32, kind="ExternalInput")
    true_cm = nc.dram_tensor("true_cm", [1, PADF + E * NPIX + PADB], f32, kind="ExternalInput")
    bin_cm = nc.dram_tensor("bin_cm", [1, PADF + Q * NPIX + PADB], f32, kind="ExternalInput")
    smalls = nc.dram_tensor("smalls", [P, SC], f32, kind="ExternalInput")
    partials = nc.dram_tensor("partials", [1, 8], f32, kind="ExternalOutput")

    def bc(ap, pos, count):
        """Insert a stride-0 broadcast dim into an AP at free-dim position pos."""
        new = list(ap.ap)
        new.insert(pos, [0, count])
        return bass.AP(tensor=ap.tensor, offset=ap.offset, ap=new)

    from contextlib import ExitStack

    with tile.TileContext(nc) as tc, ExitStack() as ctx:
        sing = ctx.enter_context(tc.tile_pool(name="sing", bufs=1))
        pkp = ctx.enter_context(tc.tile_pool(name="pkp", bufs=3))
        big = ctx.enter_context(tc.tile_pool(name="big", bufs=2))
        ps = ctx.enter_context(tc.tile_pool(name="ps", bufs=1, space="PSUM"))

        # ---------- DMA issues: chunk stream on Sync, smalls/occ on Scalar ----------
        def issue_chunk(c):
            t = pkp.tile([P, JC, CH], f32, tag="pk")
            nc.sync.dma_start(out=t[:], in_=stream.ap()[c * P:(c + 1) * P, :, :])
            return t

        sm = sing.tile([P, SC], f32)
        nc.scalar.dma_start(out=sm[:], in_=smalls.ap())
        occ_t = sing.tile([P, J, K + 1], f32)
        nc.scalar.dma_start(out=occ_t[:], in_=occ_pack.ap())
        pk_fifo = [issue_chunk(0), issue_chunk(1)]

        def S(p0, p1, c0, c1):
            return sm[p0:p1, c0:c1]

        ones = sing.tile([P, 1], f32)
        nc.vector.memset(ones[:], 1.0)
        stats = sing.tile([P, 6], f32)
        nc.vector.memset(stats[:], 0.0)
        res = sing.tile([1, 8], f32)
        nc.vector.memset(res[:], 0.0)

        # ---------- window offsets (first DVE work; needs only smalls) ----------
        ptsr = S(0, M, C_INC, C_INC + 2)
        rmag = sing.tile([M, 2], f32)
        nc.vector.tensor_scalar(out=rmag[:], in0=ptsr, scalar1=MAGIC, scalar2=-MAGIC,
                                op0=OP.add, op1=OP.add)
        gtm = sing.tile([M, 2], f32)
        nc.vector.tensor_tensor(out=gtm[:], in0=rmag[:], in1=ptsr, op=OP.is_gt)
        pixf = sing.tile([M, 2], f32)
        nc.vector.tensor_tensor(out=pixf[:], in0=rmag[:], in1=gtm[:], op=OP.subtract)
        base = sing.tile([M, 1], f32)
        nc.vector.tensor_scalar(out=base[:], in0=pixf[:, 0:1], scalar1=float(W),
                                scalar2=float(-3 * W - 3), op0=OP.mult, op1=OP.add)
        nc.vector.tensor_tensor(out=base[:], in0=base[:], in1=pixf[:, 1:2], op=OP.add)
        sofs = sing.tile([M, WIN], f32)
        nc.vector.tensor_scalar(out=sofs[:], in0=S(0, M, C_DROF, C_DROF + WIN),
                                scalar1=base[:], scalar2=S(0, M, C_RB, C_RB + 1),
                                op0=OP.add, op1=OP.add)
        v1 = sing.tile([M, WIN], f32)
        nc.vector.tensor_scalar(out=v1[:], in0=sofs[:], scalar1=0.0, scalar2=None, op0=OP.is_ge)
        v2 = sing.tile([M, WIN], f32)
        nc.vector.tensor_scalar(out=v2[:], in0=sofs[:], scalar1=float(NPIX - 1), scalar2=None, op0=OP.is_le)
        valid = sing.tile([M, WIN], f32)
        nc.vector.tensor_tensor(out=valid[:], in0=v1[:], in1=v2[:], op=OP.mult)
        # one offset per m: start of the contiguous RUNW-float span, clamped so
        # the span stays inside the padded flat tensor. Border margins mean
        # straddling windows are never clamped, so valid rows stay row-aligned.
        clam0 = sing.tile([M, 1], f32)
        nc.vector.tensor_scalar(out=clam0[:], in0=sofs[:, 0:1], scalar1=float(-(6 * W + 4)),
                                scalar2=float(NPIX - WIN), op0=OP.max, op1=OP.min)
        soft = sing.tile([M, 1], f32)
        nc.vector.tensor_scalar(out=soft[:], in0=clam0[:], scalar1=S(0, M, C_MENP, C_MENP + 1),
                                scalar2=MAGIC + PADF, op0=OP.add, op1=OP.add)
        soft_i = sing.tile([M, 1], i32)
        nc.vector.tensor_scalar(out=soft_i[:], in0=soft[:].bitcast(i32), scalar1=0x007FFFFF,
                                scalar2=None, op0=OP.bitwise_and)
        sofb = sing.tile([M, 1], f32)
        nc.vector.tensor_scalar(out=sofb[:], in0=clam0[:], scalar1=S(0, M, C_MQNP, C_MQNP + 1),
                                scalar2=MAGIC + PADF, op0=OP.add, op1=OP.add)
        sofb_i = sing.tile([M, 1], i32)
        nc.vector.tensor_scalar(out=sofb_i[:], in0=sofb[:].bitcast(i32), scalar1=0x007FFFFF,
                                scalar2=None, op0=OP.bitwise_and)

        # ---------- window gathers: one RUNW-float run per matched electron ----------
        tw = sing.tile([M, RUNW], f32)
        bw = sing.tile([M, RUNW], f32)
        true_flat = bass.AP(tensor=true_cm.ap().tensor, offset=0,
                            ap=[[1, PADF + E * NPIX + PADB], [1, 1]])
        bin_flat = bass.AP(tensor=bin_cm.ap().tensor, offset=0,
                           ap=[[1, PADF + Q * NPIX + PADB], [1, 1]])
        nc.gpsimd.indirect_dma_start(
            out=tw[:], out_offset=None, in_=true_flat,
            in_offset=bass.IndirectOffsetOnAxis(ap=soft_i[:], axis=0))
        nc.gpsimd.indirect_dma_start(
            out=bw[:], out_offset=None, in_=bin_flat,
            in_offset=bass.IndirectOffsetOnAxis(ap=sofb_i[:], axis=0))

        # ---------- dice streaming ----------
        C_ps = ps.tile([M, M], f32)
        for c in range(NCHUNK):
            pk_t = pk_fifo.pop(0)
            por_v = pk_t[:, :, 0:M]
            tru_v = pk_t[:, :, M:CH]
            exp_t = big.tile([P, JC, M], bf16, tag="exp")
            exp_i = nc.scalar.activation(out=exp_t[:], in_=por_v, func=AF.Exp)
            if c == NCHUNK - 1:
                last_exp = exp_i
            zq_t = big.tile([P, JC], f32, tag="zq")
            nc.vector.reduce_sum(out=zq_t[:], in_=exp_t[:], axis=AX.X)
            rz_t = big.tile([P, JC], bf16, tag="rz")
            with nc.allow_low_precision(reason="rz scales both num and den; error cancels in dice ratio"):
                nc.vector.reciprocal(out=rz_t[:], in_=zq_t[:])
            tc_t = big.tile([P, JC, M], bf16, tag="tc")
            nc.scalar.activation(out=tc_t[:], in_=tru_v, func=AF.Copy)
            a_t = big.tile([P, JC, M], bf16, tag="a")
            nc.vector.tensor_tensor(out=a_t[:], in0=tc_t[:], in1=bc(rz_t[:], 2, M), op=OP.mult)
            for kb in range(JC):
                nc.tensor.matmul(out=C_ps[:], lhsT=a_t[:, kb, :], rhs=exp_t[:, kb, :],
                                 start=(c == 0 and kb == 0),
                                 stop=(c == NCHUNK - 1 and kb == JC - 1))
            if c + 2 < NCHUNK:
                pk_fifo.append(issue_chunk(c + 2))

            # ---- Exp-table / DVE work slotted between chunks (Ln deferred) ----
            if c == 2:
                e4 = sing.tile([P, J, K], f32)
                nc.scalar.activation(out=e4[:], in_=occ_t[:, :, 0:K], func=AF.Exp)
                s4 = sing.tile([P, J], f32)
                nc.vector.reduce_sum(out=s4[:], in_=e4[:], axis=AX.X)
            if c == 3:
                # occupancy CE: label select
                xt = sing.tile([P, J], f32)
                mk = sing.tile([P, J], f32)
                pk2 = sing.tile([P, J], f32)
                for k in range(K):
                    nc.vector.tensor_scalar(out=mk[:], in0=occ_t[:, :, K], scalar1=float(k),
                                            scalar2=None, op0=OP.is_equal)
                    if k == 0:
                        nc.vector.tensor_tensor(out=xt[:], in0=mk[:], in1=occ_t[:, :, 0], op=OP.mult)
                    else:
                        nc.vector.tensor_tensor(out=pk2[:], in0=mk[:], in1=occ_t[:, :, k], op=OP.mult)
                        nc.vector.tensor_tensor(out=xt[:], in0=xt[:], in1=pk2[:], op=OP.add)
            if c == 4:
                # 7x7 window BCE: the 49 window values sit at run[a*W + b]
                def win_ap(t):
                    pdim = t[:].ap[0]
                    return bass.AP(tensor=t[:].tensor, offset=t[:].offset,
                                   ap=[pdim, [W, WIN], [1, WIN]])

                tv = sing.tile([M, WIN * WIN], f32)
                nc.vector.tensor_copy(out=tv[:].rearrange("m (a b) -> m a b", a=WIN),
                                      in_=win_ap(tw))
                lg = sing.tile([M, WIN * WIN], f32)
                nc.vector.tensor_copy(out=lg[:].rearrange("m (a b) -> m a b", a=WIN),
                                      in_=win_ap(bw))
                # softplus = Ln(1 + exp); the Ln half runs in the tail
                exw = sing.tile([M, WIN * WIN], f32)
                nc.scalar.activation(out=exw[:], in_=lg[:], func=AF.Exp)
                nc.vector.tensor_scalar(out=exw[:], in0=exw[:], scalar1=1.0, scalar2=None, op0=OP.add)
                prw = sing.tile([M, WIN * WIN], f32)
                nc.vector.tensor_tensor(out=prw[:], in0=lg[:], in1=tv[:], op=OP.mult)
            if c == 5:
                valid49 = sing.tile([M, WIN * WIN], f32)
                nc.vector.tensor_copy(out=valid49[:].rearrange("m (a b) -> m a b", a=WIN),
                                      in_=bc(valid[:], 2, WIN))
                iel = S(0, 1, C_IEL, C_IEL + Q)
                exc = sing.tile([1, Q], f32)
                nc.scalar.activation(out=exc[:], in_=iel, func=AF.Exp)
                nc.vector.tensor_scalar(out=exc[:], in0=exc[:], scalar1=1.0, scalar2=None, op0=OP.add)
            if c == 6:
                # NLL (96 partitions): everything except the Ln
                cenr = S(0, M, C_POS, C_POS + 2)
                chol0 = S(0, M, C_CHOL, C_CHOL + 1)
                chol1 = S(0, M, C_CHOL + 2, C_CHOL + 3)
                chol3 = S(0, M, C_CHOL + 3, C_CHOL + 4)
                d_ = sing.tile([M, 2], f32)
                nc.vector.tensor_tensor(out=d_[:], in0=ptsr, in1=cenr, op=OP.subtract)
                r00 = sing.tile([M, 1], f32)
                nc.vector.reciprocal(out=r00[:], in_=chol0)
                r11 = sing.tile([M, 1], f32)
                nc.vector.reciprocal(out=r11[:], in_=chol3)
                z0 = sing.tile([M, 1], f32)
                nc.vector.tensor_tensor(out=z0[:], in0=d_[:, 0:1], in1=r00[:], op=OP.mult)
                t1 = sing.tile([M, 1], f32)
                nc.vector.tensor_tensor(out=t1[:], in0=chol1, in1=z0[:], op=OP.mult)
                nc.vector.tensor_tensor(out=t1[:], in0=d_[:, 1:2], in1=t1[:], op=OP.subtract)
                z1 = sing.tile([M, 1], f32)
                nc.vector.tensor_tensor(out=z1[:], in0=t1[:], in1=r11[:], op=OP.mult)
                sq = sing.tile([M, 1], f32)
                nc.vector.tensor_tensor(out=sq[:], in0=z0[:], in1=z0[:], op=OP.mult)
                sq1 = sing.tile([M, 1], f32)
                nc.vector.tensor_tensor(out=sq1[:], in0=z1[:], in1=z1[:], op=OP.mult)
                nc.vector.tensor_tensor(out=sq[:], in0=sq[:], in1=sq1[:], op=OP.add)
                ldet = sing.tile([M, 1], f32)
                nc.vector.tensor_tensor(out=ldet[:], in0=chol0, in1=chol3, op=OP.mult)
                nc.vector.tensor_scalar(out=sq[:], in0=sq[:], scalar1=0.5,
                                        scalar2=float(np.log(2.0 * np.pi)), op0=OP.mult, op1=OP.add)

        # ---------- tail: all Ln work (one ACT table switch), pinned last ----------
        lse = sing.tile([P, J], f32)
        ln_i = nc.scalar.activation(out=lse[:], in_=s4[:], func=AF.Ln)
        add_dep_helper(ln_i.ins, last_exp.ins, reason="one table switch at tail")
        spw = sing.tile([M, WIN * WIN], f32)
        ln_w = nc.scalar.activation(out=spw[:], in_=exw[:], func=AF.Ln)
        add_dep_helper(ln_w.ins, last_exp.ins, reason="one table switch at tail")
        sp = sing.tile([1, Q], f32)
        ln_c = nc.scalar.activation(out=sp[:], in_=exc[:], func=AF.Ln)
        add_dep_helper(ln_c.ins, last_exp.ins, reason="one table switch at tail")
        lnd = sing.tile([M, 1], f32)
        ln_n = nc.scalar.activation(out=lnd[:], in_=ldet[:], func=AF.Ln)
        add_dep_helper(ln_n.ins, last_exp.ins, reason="one table switch at tail")

        # occupancy CE finish
        nc.vector.tensor_tensor(out=lse[:], in0=lse[:], in1=xt[:], op=OP.subtract)
        nc.vector.reduce_sum(out=stats[:, 4:5], in_=lse[:], axis=AX.X)
        # window BCE finish
        nc.vector.tensor_tensor(out=spw[:], in0=spw[:], in1=prw[:], op=OP.subtract)
        scr_w = sing.tile([M, WIN * WIN], f32)
        nc.vector.tensor_tensor(out=scr_w[:], in0=spw[:], in1=valid49[:], op=OP.mult)
        nc.vector.reduce_sum(out=stats[0:M, 1:2], in_=scr_w[:], axis=AX.X)
        # class finish
        iel = S(0, 1, C_IEL, C_IEL + Q)
        ind1 = S(0, 1, C_IND, C_IND + Q)
        t9 = sing.tile([1, Q], f32)
        nc.vector.tensor_scalar(out=t9[:], in0=sp[:], scalar1=0.9, scalar2=None, op0=OP.mult)
        nc.vector.tensor_tensor(out=t9[:], in0=t9[:], in1=iel, op=OP.subtract)
        scr_q = sing.tile([1, Q], f32)
        clsm = sing.tile([1, 1], f32)
        nc.vector.tensor_tensor(out=scr_q[:], in0=t9[:], in1=ind1, op=OP.mult)
        nc.vector.reduce_sum(out=clsm[:], in_=scr_q[:], axis=AX.X)
        spsum = sing.tile([1, 1], f32)
        nc.vector.reduce_sum(out=spsum[:], in_=sp[:], axis=AX.X)
        nc.vector.tensor_scalar(out=spsum[:], in0=spsum[:], scalar1=NO_E, scalar2=None, op0=OP.mult)
        nc.vector.tensor_tensor(out=res[:, 6:7], in0=spsum[:], in1=clsm[:], op=OP.add)
        # NLL finish
        nc.vector.tensor_tensor(out=stats[0:M, 0:1], in0=sq[:], in1=lnd[:], op=OP.add)

        # ---------- dice epilogue ----------
        Cs = sing.tile([M, M], f32)
        nc.vector.tensor_copy(out=Cs[:], in_=C_ps[:])
        # rhs rows (exp_r/Z) sum to 1, so summing all of C gives sum(true): den.
        nc.vector.reduce_sum(out=stats[0:M, 3:4], in_=Cs[:], axis=AX.X)
        scr_c = sing.tile([M, M], f32)
        nc.vector.tensor_tensor(out=scr_c[:], in0=Cs[:], in1=S(0, M, C_I96, C_I96 + M), op=OP.mult)
        nc.vector.reduce_sum(out=stats[0:M, 2:3], in_=scr_c[:], axis=AX.X)

        # ---------- final cross-partition reduction ----------
        fin_ps = ps.tile([1, 6], f32)
        nc.tensor.matmul(out=fin_ps[:], lhsT=ones[:], rhs=stats[:], start=True, stop=True)
        nc.vector.tensor_copy(out=res[:, 0:6], in_=fin_ps[:])
        nc.sync.dma_start(out=partials.ap(), in_=res[:])

    nc.compile()
    return nc


def _get_nc():
    if "nc" not in _CACHE:
        _CACHE["nc"] = _build_nc()
    return _CACHE["nc"]


def make_in_maps(is_electron_logit, true_segmap, binary_mask_logits, portion_logits,
                 incidence_points, positions, chol, occupancy_logits, occupancy_true,
                 matched_q, matched_e):
    f = np.float32
    in_maps = []
    for c in range(8):
        b, h = c // 2, c % 2
        sl = slice(h * HALF, (h + 1) * HALF)
        me = np.asarray(matched_e[b])
        mq = np.asarray(matched_q[b])
        true_sl = np.ascontiguousarray(true_segmap[b, sl], dtype=f).reshape(NPIX, E)
        por_sl = np.ascontiguousarray(portion_logits[b, sl], dtype=f).reshape(NPIX, Q)
        bin_sl = np.ascontiguousarray(binary_mask_logits[b, sl], dtype=f).reshape(NPIX, Q)
        # channel gathers: pure indexing (reference's take_along_axis layout)
        stream = np.concatenate([por_sl[:, mq], true_sl[:, me]], axis=1)
        stream = np.ascontiguousarray(stream).reshape(NCHUNK * P, JC, CH)
        occ_sl = np.asarray(occupancy_logits[b, sl], dtype=f).reshape(P, J, K)
        occt = np.asarray(occupancy_true[b, sl], dtype=f).reshape(P, J, 1)
        occ_pack = np.concatenate([occ_sl, occt], axis=2)

        sm = np.zeros((P, SC), dtype=f)
        sm[:M, C_RB] = -h * NPIX
        sm[:M, C_INC:C_INC + 2] = np.asarray(incidence_points[b], dtype=f)[me]
        sm[:M, C_DROF:C_DROF + WIN] = np.tile(np.arange(WIN, dtype=f) * W, (M, 1))
        sm[:M, C_POS:C_POS + 2] = np.asarray(positions[b], dtype=f)[mq]
        sm[:M, C_CHOL:C_CHOL + 4] = np.asarray(chol[b], dtype=f).reshape(Q, 4)[mq]
        sm[:M, C_MENP] = me.astype(f) * NPIX
        sm[:M, C_MQNP] = mq.astype(f) * NPIX
        sm[0, C_IEL:C_IEL + Q] = np.asarray(is_electron_logit, dtype=f).reshape(B, Q)[b]
        ind = np.zeros(Q, dtype=f)
        ind[mq] = 1.0
        sm[0, C_IND:C_IND + Q] = ind
        sm[:M, C_I96:C_I96 + M] = np.eye(M, dtype=f)

        def flat_pad(cm):
            out = np.zeros((1, PADF + cm.size + PADB), dtype=f)
            out[0, PADF:PADF + cm.size] = cm.reshape(-1)
            return out

        in_maps.append(dict(
            stream=stream,
            occ_pack=occ_pack,
            true_cm=flat_pad(np.ascontiguousarray(true_sl.T)),
            bin_cm=flat_pad(np.ascontiguousarray(bin_sl.T)),
            smalls=sm,
        ))
    return in_maps


def combine(partials_list):
    s = np.stack([np.asarray(p, dtype=np.float64).reshape(8) for p in partials_list])
    # slots: 0=nll_sum 1=bce_sum 2=num2_sum 3=den_true_sum 4=occ_sum 6=class_sum
    class_loss = s[0::2, 6].sum() / (B * Q)
    nll_loss = s[0::2, 0].sum() / (B * M)
    bce_loss = s[:, 1].sum() / (B * M * WIN * WIN)
    occ_loss = s[:, 4].sum() / (B * H * W)
    dice = 0.0
    for b in range(B):
        num = 2.0 * (s[2 * b, 2] + s[2 * b + 1, 2])
        den = s[2 * b, 3] + s[2 * b + 1, 3] + H * W
        dice += 1.0 - (num + 1.0) / (den + 1.0)
    dice_loss = dice / B
    return np.float32(class_loss + bce_loss + dice_loss + nll_loss + occ_loss)


def kernel(**inputs):
    from concourse.bass_utils import run_bass_kernel_spmd
    nc = _get_nc()
    in_maps = make_in_maps(**{k: np.asarray(v) for k, v in inputs.items()})
    r = run_bass_kernel_spmd(nc, in_maps, list(range(8)))
    return combine([r.results[c]["partials"] for c in range(8)])
